# revision 9
# baseline (speedup 1.0000x reference)
"""CPSF codebook fused kernel for 8 Trainium2 NeuronCores.

Math (see reference): for each batch row b and codebook entry m,
  q[b,m] = par_sq/s_par + (max(tot_sq-par_sq,0) + max(dd_sq,0))/s_perp
  w[b,m] = alpha[m] * exp(-pi*q)
  out    = Re((w @ (T_hat_re + i*T_hat_im)) @ A.T),  A = exp(i*2pi/S * k*s)

Device strategy (M x B grid over 8 cores, host-side reduction):
  - The final DFT is folded into the codebook on the host:
      out = w @ TA,  TA = T_hat_re @ cos(ang) - T_hat_im @ sin(ang)
  - ALL linear and constant terms of the exponent are folded into the
    codebook-side matmul stacks on the host, so the device evaluates
      w[b,m] = exp( sgn*(M1^2 + M2^2) + S + be[m] )
    with M1/M2/S raw matmul outputs. Per 128m x 512b tile that is:
    4 weight matmuls + 2 out matmuls (PE), one Square (ACT), one
    self-multiply (DVE), one add (GpSimd), one PSUM-combine (DVE),
    one Exp (ACT) -- every engine under the PE's ~1.4us/tile pace.
  - Cores form a 2x4 grid: m-half x b-quarter. Each core computes
    partial out.T[256, 1024] for its quarter of the batch over half the
    codebook; the host adds the two m-halves, transposes, and applies
    the per-row exp factor. The relu clamps are dropped: tot_sq-par_sq
    >= 0 and dd_sq >= 0 hold mathematically; clamping only trims float
    roundoff ~1e-6, far below output noise.
"""

import os
import sys

for _p in ("/opt/trn_rl_repo", os.path.expanduser("~/.axon_site/_ro/trn_rl_repo")):
    if os.path.isdir(_p) and _p not in sys.path:
        sys.path.insert(0, _p)

import numpy as np

B, N, M, S = 4096, 64, 8192, 256
NCORES = 8
MG = 2                      # m-groups (codebook halves)
BGN = 4                     # b-groups (batch quarters)
MLOC = M // MG              # 4096 codebook entries per core
NT = MLOC // 128            # 32 m-tiles per core
BLOC = B // BGN             # 1024 batch rows per core
CW = 512                    # PSUM tile width (batch cols per chunk)
NCH = BLOC // CW            # 2 chunks per core
KSH = 5                     # out-matmul software pipeline shift (tiles)
PI = float(np.pi)

SELF_TT = True              # DVE tensor_mul(out, P, P) with P in PSUM


def _prep(x_re, x_im, z_j_re, z_j_im, vec_d_j_re, vec_d_j_im,
          T_hat_re, T_hat_im, alpha_j, sigma_par, sigma_perp):
    """Host-side operand packing (all O(B*N + M*N + M*S^2) — tiny vs device)."""
    f32 = np.float32
    f64 = np.float64
    tiny = np.finfo(f32).tiny

    # ---- batch side ----
    z_re = np.ascontiguousarray(x_re[:, :N]).astype(f32)
    z_im = np.ascontiguousarray(x_im[:, :N]).astype(f32)
    vd_re = np.ascontiguousarray(x_re[:, N:]).astype(f32)
    vd_im = np.ascontiguousarray(x_im[:, N:]).astype(f32)
    nrm = np.sqrt((vd_re * vd_re + vd_im * vd_im).sum(-1, dtype=f32)).astype(f32)
    nrm = np.where(nrm == 0, f32(1.0), nrm)
    vd_re = vd_re / nrm[:, None]
    vd_im = vd_im / nrm[:, None]
    z_sq = (z_re * z_re + z_im * z_im).sum(-1, dtype=f32)
    vd_sq = (vd_re * vd_re + vd_im * vd_im).sum(-1, dtype=f32)

    r1 = np.ascontiguousarray(np.concatenate([z_re.T, z_im.T], 0))      # [128, B]
    r2 = np.ascontiguousarray(np.concatenate([vd_re.T, vd_im.T], 0))    # [128, B]

    # ---- codebook side ----
    djr = z_j_re.astype(f32)
    dji = z_j_im.astype(f32)
    vr = vec_d_j_re.astype(f32)
    vi = vec_d_j_im.astype(f32)
    nj = np.sqrt((vr * vr + vi * vi).sum(-1, dtype=f32)).astype(f32)
    nj = np.where(nj == 0, f32(1.0), nj)
    vr = vr / nj[:, None]
    vi = vi / nj[:, None]

    alpha = np.maximum(alpha_j.astype(f32), tiny).astype(f64)
    s_par = np.maximum(sigma_par.astype(f32), tiny).astype(f64)
    s_perp = np.maximum(sigma_perp.astype(f32), tiny).astype(f64)
    inv_sp = 1.0 / s_perp
    dbar = inv_sp - 1.0 / s_par
    gam = np.sqrt(PI * np.abs(dbar))
    sgn = np.where(dbar >= 0, 1.0, -1.0)

    c0_re = ((vr * djr + vi * dji).astype(f64)).sum(-1)
    c0_im = ((vr * dji - vi * djr).astype(f64)).sum(-1)
    z_j_sq = ((djr * djr + dji * dji).astype(f64)).sum(-1)
    vd_j_sq = ((vr * vr + vi * vi).astype(f64)).sum(-1)

    a1 = np.concatenate([vr.T, vi.T], 0).astype(f64)      # [128, M]
    a2 = np.concatenate([-vi.T, vr.T], 0).astype(f64)
    zc = np.concatenate([djr.T, dji.T], 0).astype(f64)

    L1 = (gam[None, :] * a1).astype(f32)
    L2 = (gam[None, :] * a2).astype(f32)
    L3 = ((2.0 * PI * inv_sp)[None, :] * zc
          - (2.0 * PI * dbar)[None, :] * (c0_re[None, :] * a1
                                          + c0_im[None, :] * a2)).astype(f32)
    L4 = ((2.0 * PI * inv_sp)[None, :] * a1).astype(f32)
    # be is folded into the out-side codebook: TA' = e^be * TA, so the
    # device exp needs no bias and can run on two-tile [128,1024] pairs.
    be = (np.log(alpha) - PI * inv_sp * (z_j_sq + vd_j_sq)
          + PI * dbar * (c0_re * c0_re + c0_im * c0_im))

    # sgn mode: 1 => plain add; -1 => stt with -1; 0 => per-m AP
    if np.all(sgn > 0):
        sgn_mode = 1
    elif np.all(sgn < 0):
        sgn_mode = -1
    else:
        sgn_mode = 0

    # v[b] term: inv_sp = c0v + delta; c0v*v[b] factors out of the exp as a
    # per-b output row scale (exact); a nonzero delta needs a rank-1 matmul.
    c0v = float(inv_sp.mean())
    delta = (inv_sp - c0v).astype(f32)
    uniform = bool(np.all(delta == 0))
    vraw = (z_sq + vd_sq).astype(f64)
    erow = np.exp(-PI * c0v * vraw)                       # [B] f64 row scale
    vrow = (z_sq + vd_sq).astype(f32)[None, :]            # [1, B]
    ispk = (-PI * delta).astype(f32)[None, :]             # [1, M]

    # DFT folded into the codebook (same fp32 angles as reference), then
    # scaled by e^be per row.
    nn = np.arange(S, dtype=f32)
    ang = f32(2.0 * np.pi / S) * (nn[:, None] * nn[None, :])
    cosA = np.cos(ang).astype(f32)
    sinA = np.sin(ang).astype(f32)
    TA = ((T_hat_re.astype(f64) @ cosA.astype(f64)
           - T_hat_im.astype(f64) @ sinA.astype(f64))
          * np.exp(be)[:, None]).astype(f32)

    # pack per m-group/m-tile: lpack [MG, NT, 128, 768] = [L1|L2|L3|L4|TA']
    lpack = np.empty((MG, NT, 128, 768), f32)
    sg = np.empty((MG, 128, NT), f32)
    for g in range(MG):
        for t in range(NT):
            sl = slice(g * MLOC + t * 128, g * MLOC + (t + 1) * 128)
            lpack[g, t, :, 0:128] = L1[:, sl]
            lpack[g, t, :, 128:256] = L2[:, sl]
            lpack[g, t, :, 256:384] = L3[:, sl]
            lpack[g, t, :, 384:512] = L4[:, sl]
            lpack[g, t, :, 512:768] = TA[sl]
            sg[g, :, t] = sgn[sl].astype(f32)

    return dict(r1=r1, r2=r2, lpack=lpack, sg=sg, vrow=vrow,
                ispk=ispk, erow=erow, uniform=uniform, sgn_mode=sgn_mode)


_CACHED = {}


def _build_nc(uniform, sgn_mode):
    key = ("nc", uniform, sgn_mode, SELF_TT)
    if key in _CACHED:
        return _CACHED[key]
    import concourse.bacc as bacc
    import concourse.mybir as mybir
    import concourse.tile as tile

    F32 = mybir.dt.float32
    F32R = mybir.dt.float32r
    AF = mybir.ActivationFunctionType
    OP = mybir.AluOpType

    nc = bacc.Bacc("TRN2", target_bir_lowering=False, debug=False,
                   num_devices=NCORES)
    d_r1 = nc.dram_tensor("r1", [128, BLOC], F32R, kind="ExternalInput").ap()
    d_r2 = nc.dram_tensor("r2", [128, BLOC], F32R, kind="ExternalInput").ap()
    d_lp = nc.dram_tensor("lp", [NT, 128, 768], F32R, kind="ExternalInput").ap()
    if sgn_mode == 0:
        d_sg = nc.dram_tensor("sg", [128, NT], F32, kind="ExternalInput").ap()
    if not uniform:
        d_isp = nc.dram_tensor("isp", [1, MLOC], F32R, kind="ExternalInput").ap()
        d_v = nc.dram_tensor("vrow", [1, BLOC], F32R, kind="ExternalInput").ap()
    d_out = nc.dram_tensor("out", [256, BLOC], F32, kind="ExternalOutput").ap()

    with tile.TileContext(nc) as tc:
        with tc.tile_pool(name="const", bufs=1) as cp, \
             tc.tile_pool(name="pp", bufs=2, space="PSUM") as pp, \
             tc.tile_pool(name="sp", bufs=2, space="PSUM") as sp, \
             tc.tile_pool(name="otp", bufs=1, space="PSUM") as otp, \
             tc.tile_pool(name="u12p", bufs=3) as u12p, \
             tc.tile_pool(name="up", bufs=3) as up, \
             tc.tile_pool(name="sfp", bufs=3) as sfp, \
             tc.tile_pool(name="wp", bufs=4) as wp, \
             tc.tile_pool(name="obsp", bufs=2) as obsp:
            r1 = cp.tile([128, BLOC], F32R)
            r2 = cp.tile([128, BLOC], F32R)
            lps = [cp.tile([128, 768], F32R, name=f"lp{t}") for t in range(NT)]
            if sgn_mode == 0:
                sg = cp.tile([128, NT], F32)
            if not uniform:
                isp = cp.tile([1, MLOC], F32R)
                vrow = cp.tile([1, BLOC], F32R)

            # initial DMA order: first-needed first; all on the sync queue
            for h in range(2):
                nc.sync.dma_start(lps[0][:, h * 384:(h + 1) * 384],
                                  d_lp[0][:, h * 384:(h + 1) * 384])
            nc.sync.dma_start(r1[:, 0:CW], d_r1[:, 0:CW])
            nc.sync.dma_start(r2[:, 0:CW], d_r2[:, 0:CW])
            if sgn_mode == 0:
                nc.sync.dma_start(sg[:], d_sg)
            if not uniform:
                nc.sync.dma_start(isp[:], d_isp)
                nc.sync.dma_start(vrow[:], d_v)
            for t in range(1, NT):
                for h in range(2):
                    nc.sync.dma_start(lps[t][:, h * 384:(h + 1) * 384],
                                      d_lp[t][:, h * 384:(h + 1) * 384])
                if t == 4:
                    nc.sync.dma_start(r1[:, CW:BLOC], d_r1[:, CW:BLOC])
                    nc.sync.dma_start(r2[:, CW:BLOC], d_r2[:, CW:BLOC])

            for ch in range(NCH):
                csl = slice(ch * CW, (ch + 1) * CW)
                sfps = {}
                wps = {}

                def emit_out(k, ot, ws=wps):
                    wpair = ws[k // 2]
                    wsl = slice((k % 2) * CW, (k % 2) * CW + CW)
                    for h in range(2):
                        nc.tensor.matmul(
                            ot[:, h * CW:(h + 1) * CW],
                            lps[k][:, 512 + h * 128:512 + (h + 1) * 128],
                            wpair[:, wsl], start=(k == 0), stop=(k == NT - 1),
                            skip_group_check=True)

                ot = otp.tile([128, 2 * CW], F32, tag="ot")
                for t in range(NT):
                    p12 = pp.tile([128, 2 * CW], F32, tag="P12")
                    sb = sp.tile([128, CW], F32, tag="S")
                    lp = lps[t]
                    nc.tensor.matmul(p12[:, 0:CW], lp[:, 0:128], r1[:, csl],
                                     start=True, stop=True)
                    nc.tensor.matmul(p12[:, CW:2 * CW], lp[:, 128:256],
                                     r1[:, csl], start=True, stop=True)
                    nc.tensor.matmul(sb[:], lp[:, 256:384], r1[:, csl],
                                     start=True, stop=False)
                    nc.tensor.matmul(sb[:], lp[:, 384:512], r2[:, csl],
                                     start=False, stop=uniform)
                    if not uniform:
                        nc.tensor.matmul(sb[:], isp[:, t * 128:(t + 1) * 128],
                                         vrow[:, csl], start=False, stop=True)
                    if t >= KSH:
                        emit_out(t - KSH, ot)

                    # both squares of this tile in one [128,1024] ACT op
                    u12 = u12p.tile([128, 2 * CW], F32, tag="u12")
                    nc.scalar.activation(u12[:], p12[:], AF.Square)
                    if t % 2 == 1 and t >= 3:
                        q = (t - 3) // 2
                        nc.scalar.activation(wps[q][:], sfps[q][:], AF.Exp)
                    # u = u1 + u2 (SBUF-only, on GpSimd)
                    u = up.tile([128, CW], F32, tag="u")
                    if sgn_mode == -1:
                        nc.gpsimd.scalar_tensor_tensor(
                            u[:], u12[:, 0:CW], -1.0, u12[:, CW:2 * CW],
                            op0=OP.mult, op1=OP.subtract)
                    else:
                        nc.gpsimd.tensor_add(u[:], u12[:, 0:CW],
                                             u12[:, CW:2 * CW])
                    # sf = sgn*u + S into the pair tile half (DVE)
                    if t % 2 == 0:
                        sfps[t // 2] = sfp.tile([128, 2 * CW], F32, tag="sf",
                                                name=f"sf{ch}_{t // 2}")
                        wps[t // 2] = wp.tile([128, 2 * CW], F32R, tag="w",
                                              name=f"w{ch}_{t // 2}")
                    sfh = sfps[t // 2][:, (t % 2) * CW:(t % 2) * CW + CW]
                    if sgn_mode == 0:
                        nc.vector.scalar_tensor_tensor(
                            sfh, u[:], sg[:, t:t + 1], sb[:],
                            op0=OP.mult, op1=OP.add)
                    else:
                        nc.vector.tensor_add(sfh, u[:], sb[:])

                # drain the per-chunk pipeline (pairs 0..(NT-3)//2 emitted
                # in-loop at odd tiles)
                nc.scalar.activation(wps[NT // 2 - 1][:],
                                     sfps[NT // 2 - 1][:], AF.Exp)
                for k in range(NT - KSH, NT):
                    emit_out(k, ot)

                obs = obsp.tile([128, 2 * CW], F32, tag="obs")
                nc.vector.tensor_copy(obs[:], ot[:])
                for h in range(2):
                    nc.sync.dma_start(d_out[h * 128:(h + 1) * 128, csl],
                                      obs[:, h * CW:(h + 1) * CW])
    nc.compile()
    _CACHED[key] = nc
    return nc


def _run(inputs, trace=False):
    from concourse.bass_utils import run_bass_kernel_spmd

    prep = _prep(**inputs)
    nc = _build_nc(prep["uniform"], prep["sgn_mode"])
    in_maps = []
    for c in range(NCORES):
        g, bg = c // BGN, c % BGN
        bsl = slice(bg * BLOC, (bg + 1) * BLOC)
        im = dict(r1=np.ascontiguousarray(prep["r1"][:, bsl]),
                  r2=np.ascontiguousarray(prep["r2"][:, bsl]),
                  lp=prep["lpack"][g])
        if prep["sgn_mode"] == 0:
            im["sg"] = np.ascontiguousarray(prep["sg"][g])
        if not prep["uniform"]:
            im["isp"] = np.ascontiguousarray(
                prep["ispk"][:, g * MLOC:(g + 1) * MLOC])
            im["vrow"] = np.ascontiguousarray(prep["vrow"][:, bsl])
        in_maps.append(im)
    res = run_bass_kernel_spmd(nc, in_maps, list(range(NCORES)), trace=trace)
    out = np.empty((B, S), np.float64)
    for bg in range(BGN):
        sm = (res.results[bg]["out"].astype(np.float64)
              + res.results[BGN + bg]["out"].astype(np.float64))
        bsl = slice(bg * BLOC, (bg + 1) * BLOC)
        out[bsl] = sm.T * prep["erow"][bsl, None]
    return out.astype(np.float32), res


def kernel(**inputs):
    out, _ = _run(inputs, trace=False)
    return out


def _install_ntff_hook():
    """The agent image's antenv lacks axon_hooks; recreate it so trace=True
    can capture NTFF profiles via libaxon_pjrt.so (same mechanism as
    trn_agent_boot.trn_boot)."""
    import types

    try:
        from antenv.axon_hooks import get_axon_ntff_profile_hook  # noqa: F401
        return
    except ImportError:
        pass
    import contextlib
    import ctypes

    so_path = "/opt/axon/libaxon_pjrt.so"
    lib = ctypes.CDLL(so_path)
    lib.axon_start_nrt_profile.argtypes = [ctypes.POINTER(ctypes.c_int64),
                                           ctypes.c_size_t]
    lib.axon_start_nrt_profile.restype = ctypes.c_int64
    lib.axon_stop_nrt_profile.argtypes = [ctypes.c_char_p]
    lib.axon_stop_nrt_profile.restype = ctypes.c_int64

    @contextlib.contextmanager
    def _hook(output_dir, device_ids):
        import jax

        jax.devices()
        if device_ids:
            ids = (ctypes.c_int64 * len(device_ids))(*device_ids)
            rc = lib.axon_start_nrt_profile(ids, len(device_ids))
        else:
            rc = lib.axon_start_nrt_profile(None, 0)
        if rc != 0:
            raise RuntimeError(f"axon_start_nrt_profile rc={rc}")
        try:
            yield
        finally:
            n = lib.axon_stop_nrt_profile(str(output_dir).encode())
            if n < 0:
                raise RuntimeError(f"axon_stop_nrt_profile rc={n}")
            if n == 0:
                print("WARNING: NTFF capture wrote nothing (raced the execute)")

    mod = types.ModuleType("antenv.axon_hooks")
    mod.get_axon_ntff_profile_hook = lambda: _hook
    mod.set_axon_ntff_profile_hook = lambda h: None
    sys.modules["antenv.axon_hooks"] = mod
    import antenv

    antenv.axon_hooks = mod


def run_traced(inputs):
    _install_ntff_hook()
    return _run(inputs, trace=True)


# revision 22
# speedup vs baseline: 1.2142x; 1.2142x over previous
"""CPSF codebook fused kernel for 8 Trainium2 NeuronCores.

Math (see reference): for each batch row b and codebook entry m,
  q[b,m] = par_sq/s_par + (max(tot_sq-par_sq,0) + max(dd_sq,0))/s_perp
  w[b,m] = alpha[m] * exp(-pi*q)
  out    = Re((w @ (T_hat_re + i*T_hat_im)) @ A.T),  A = exp(i*2pi/S * k*s)

Device strategy (M x B grid over 8 cores, host-side reduction):
  - The final DFT is folded into the codebook on the host:
      out = w @ TA,  TA = T_hat_re @ cos(ang) - T_hat_im @ sin(ang)
  - ALL linear and constant terms of the exponent are folded into the
    codebook-side matmul stacks on the host, so the device evaluates
      w[b,m] = exp( sgn*(M1^2 + M2^2) + S + be[m] )
    with M1/M2/S raw matmul outputs. Per 128m x 512b tile that is:
    4 weight matmuls + 2 out matmuls (PE), one Square (ACT), one
    self-multiply (DVE), one add (GpSimd), one PSUM-combine (DVE),
    one Exp (ACT) -- every engine under the PE's ~1.4us/tile pace.
  - Cores form a 2x4 grid: m-half x b-quarter. Each core computes
    partial out.T[256, 1024] for its quarter of the batch over half the
    codebook; the host adds the two m-halves, transposes, and applies
    the per-row exp factor. The relu clamps are dropped: tot_sq-par_sq
    >= 0 and dd_sq >= 0 hold mathematically; clamping only trims float
    roundoff ~1e-6, far below output noise.
"""

import os
import sys

for _p in ("/opt/trn_rl_repo", os.path.expanduser("~/.axon_site/_ro/trn_rl_repo")):
    if os.path.isdir(_p) and _p not in sys.path:
        sys.path.insert(0, _p)

import numpy as np

B, N, M, S = 4096, 64, 8192, 256
NCORES = 8
MG = 2                      # m-groups (codebook halves)
BGN = 4                     # b-groups (batch quarters)
MLOC = M // MG              # 4096 codebook entries per core
NT = MLOC // 128            # 32 m-tiles per core
BLOC = B // BGN             # 1024 batch rows per core
CW = 512                    # PSUM tile width (batch cols per chunk)
NCH = BLOC // CW            # 2 chunks per core
KSH = 5                     # out-matmul software pipeline shift (tiles)
PI = float(np.pi)

OFFC = 224                  # square columns offloaded from ACT to DVE copy+mult
XW = 2 * CW - OFFC          # ACT square width


def _prep(x_re, x_im, z_j_re, z_j_im, vec_d_j_re, vec_d_j_im,
          T_hat_re, T_hat_im, alpha_j, sigma_par, sigma_perp):
    """Host-side operand packing (all O(B*N + M*N + M*S^2) — tiny vs device)."""
    f32 = np.float32
    f64 = np.float64
    tiny = np.finfo(f32).tiny

    # ---- batch side ----
    z_re = np.ascontiguousarray(x_re[:, :N]).astype(f32)
    z_im = np.ascontiguousarray(x_im[:, :N]).astype(f32)
    vd_re = np.ascontiguousarray(x_re[:, N:]).astype(f32)
    vd_im = np.ascontiguousarray(x_im[:, N:]).astype(f32)
    nrm = np.sqrt((vd_re * vd_re + vd_im * vd_im).sum(-1, dtype=f32)).astype(f32)
    nrm = np.where(nrm == 0, f32(1.0), nrm)
    vd_re = vd_re / nrm[:, None]
    vd_im = vd_im / nrm[:, None]
    z_sq = (z_re * z_re + z_im * z_im).sum(-1, dtype=f32)
    vd_sq = (vd_re * vd_re + vd_im * vd_im).sum(-1, dtype=f32)

    import ml_dtypes
    bf = ml_dtypes.bfloat16
    # batch-side operands in bf16 to match the bf16 codebook stacks
    # (matmul inputs must be same width)
    r1 = np.ascontiguousarray(np.concatenate([z_re.T, z_im.T], 0)).astype(bf)
    r2 = np.ascontiguousarray(np.concatenate([vd_re.T, vd_im.T], 0)).astype(bf)

    # ---- codebook side ----
    djr = z_j_re.astype(f32)
    dji = z_j_im.astype(f32)
    vr = vec_d_j_re.astype(f32)
    vi = vec_d_j_im.astype(f32)
    nj = np.sqrt((vr * vr + vi * vi).sum(-1, dtype=f32)).astype(f32)
    nj = np.where(nj == 0, f32(1.0), nj)
    vr = vr / nj[:, None]
    vi = vi / nj[:, None]

    alpha = np.maximum(alpha_j.astype(f32), tiny).astype(f64)
    s_par = np.maximum(sigma_par.astype(f32), tiny).astype(f64)
    s_perp = np.maximum(sigma_perp.astype(f32), tiny).astype(f64)
    inv_sp = 1.0 / s_perp
    dbar = inv_sp - 1.0 / s_par
    gam = np.sqrt(PI * np.abs(dbar))
    sgn = np.where(dbar >= 0, 1.0, -1.0)

    c0_re = ((vr * djr + vi * dji).astype(f64)).sum(-1)
    c0_im = ((vr * dji - vi * djr).astype(f64)).sum(-1)
    z_j_sq = ((djr * djr + dji * dji).astype(f64)).sum(-1)
    vd_j_sq = ((vr * vr + vi * vi).astype(f64)).sum(-1)

    a1 = np.concatenate([vr.T, vi.T], 0).astype(f64)      # [128, M]
    a2 = np.concatenate([-vi.T, vr.T], 0).astype(f64)
    zc = np.concatenate([djr.T, dji.T], 0).astype(f64)

    L1 = (gam[None, :] * a1).astype(f32)
    L2 = (gam[None, :] * a2).astype(f32)
    L3 = ((2.0 * PI * inv_sp)[None, :] * zc
          - (2.0 * PI * dbar)[None, :] * (c0_re[None, :] * a1
                                          + c0_im[None, :] * a2)).astype(f32)
    L4 = ((2.0 * PI * inv_sp)[None, :] * a1).astype(f32)
    # be is folded into the out-side codebook: TA' = e^be * TA, so the
    # device exp needs no bias and can run on two-tile [128,1024] pairs.
    be = (np.log(alpha) - PI * inv_sp * (z_j_sq + vd_j_sq)
          + PI * dbar * (c0_re * c0_re + c0_im * c0_im))

    # sgn mode: 1 => plain add; -1 => stt with -1; 0 => per-m AP
    if np.all(sgn > 0):
        sgn_mode = 1
    elif np.all(sgn < 0):
        sgn_mode = -1
    else:
        sgn_mode = 0

    # v[b] term: inv_sp = c0v + delta; c0v*v[b] factors out of the exp as a
    # per-b output row scale (exact); a nonzero delta needs a rank-1 matmul.
    # Threshold absorbs mean-rounding dust (exp-arg error < 1e-12).
    c0v = float(inv_sp.mean())
    delta = (inv_sp - c0v).astype(f32)
    uniform = bool(np.abs(delta).max() <= 1e-9 * abs(c0v))
    vraw = (z_sq + vd_sq).astype(f64)
    erow = np.exp(-PI * c0v * vraw)                       # [B] f64 row scale
    vrow = (z_sq + vd_sq).astype(f32)[None, :]            # [1, B]
    ispk = (-PI * delta).astype(f32)[None, :]             # [1, M]

    # DFT folded into the codebook (same fp32 angles as reference), then
    # scaled by e^be per row.
    nn = np.arange(S, dtype=f32)
    ang = f32(2.0 * np.pi / S) * (nn[:, None] * nn[None, :])
    cosA = np.cos(ang).astype(f32)
    sinA = np.sin(ang).astype(f32)
    TA = ((T_hat_re.astype(f64) @ cosA.astype(f64)
           - T_hat_im.astype(f64) @ sinA.astype(f64))
          * np.exp(be)[:, None]).astype(f32)

    # pack per m-group/m-tile: lpack [MG, NT, 128, 768] = [L1|L2|L3|L4|TA'],
    # stored bf16 (halves LDWEIGHTS + DMA; ~3e-3 output rel err, gate 2e-2)
    lpack = np.empty((MG, NT, 128, 768), bf)
    sg = np.empty((MG, 128, NT), f32)
    for g in range(MG):
        for t in range(NT):
            sl = slice(g * MLOC + t * 128, g * MLOC + (t + 1) * 128)
            lpack[g, t, :, 0:128] = L1[:, sl]
            lpack[g, t, :, 128:256] = L2[:, sl]
            lpack[g, t, :, 256:384] = L3[:, sl]
            lpack[g, t, :, 384:512] = L4[:, sl]
            lpack[g, t, :, 512:768] = TA[sl]
            sg[g, :, t] = sgn[sl].astype(f32)

    return dict(r1=r1, r2=r2, lpack=lpack, sg=sg, vrow=vrow,
                ispk=ispk, erow=erow, uniform=uniform, sgn_mode=sgn_mode)


_CACHED = {}


def _build_nc(uniform, sgn_mode):
    key = ("nc", uniform, sgn_mode)
    if key in _CACHED:
        return _CACHED[key]
    import concourse.bacc as bacc
    import concourse.mybir as mybir
    import concourse.tile as tile

    F32 = mybir.dt.float32
    F32R = mybir.dt.float32r
    BF16 = mybir.dt.bfloat16
    AF = mybir.ActivationFunctionType
    OP = mybir.AluOpType

    nc = bacc.Bacc("TRN2", target_bir_lowering=False, debug=False,
                   num_devices=NCORES)
    d_r1 = nc.dram_tensor("r1", [128, BLOC], BF16, kind="ExternalInput").ap()
    d_r2 = nc.dram_tensor("r2", [128, BLOC], BF16, kind="ExternalInput").ap()
    d_lp = nc.dram_tensor("lp", [NT, 128, 768], BF16, kind="ExternalInput").ap()
    if sgn_mode == 0:
        d_sg = nc.dram_tensor("sg", [128, NT], F32, kind="ExternalInput").ap()
    if not uniform:
        d_isp = nc.dram_tensor("isp", [1, MLOC], F32R, kind="ExternalInput").ap()
        d_v = nc.dram_tensor("vrow", [1, BLOC], F32R, kind="ExternalInput").ap()
    d_out = nc.dram_tensor("out", [256, BLOC], F32, kind="ExternalOutput").ap()

    with tile.TileContext(nc) as tc:
        with tc.tile_pool(name="const", bufs=1) as cp, \
             tc.tile_pool(name="pp", bufs=2, space="PSUM") as pp, \
             tc.tile_pool(name="sp", bufs=2, space="PSUM") as sp, \
             tc.tile_pool(name="otp", bufs=1, space="PSUM") as otp, \
             tc.tile_pool(name="u12p", bufs=3) as u12p, \
             tc.tile_pool(name="ccp", bufs=3) as ccp, \
             tc.tile_pool(name="t1p", bufs=3) as t1p, \
             tc.tile_pool(name="sfp", bufs=3) as sfp, \
             tc.tile_pool(name="wp", bufs=4) as wp, \
             tc.tile_pool(name="obsp", bufs=2) as obsp:
            r1 = cp.tile([128, BLOC], BF16)
            r2 = cp.tile([128, BLOC], BF16)
            lps = [cp.tile([128, 768], BF16, name=f"lp{t}") for t in range(NT)]
            if sgn_mode == 0:
                sg = cp.tile([128, NT], F32)
            if not uniform:
                isp = cp.tile([1, MLOC], F32R)
                vrow = cp.tile([1, BLOC], F32R)

            # initial DMAs split across the two HWDGE queues (sync + scalar;
            # scalar is idle until the first Square) so the first tile's
            # operands land fast
            nc.sync.dma_start(lps[0][:], d_lp[0])
            nc.scalar.dma_start(r1[:, 0:CW], d_r1[:, 0:CW])
            nc.sync.dma_start(r2[:, 0:CW], d_r2[:, 0:CW])
            if sgn_mode == 0:
                nc.scalar.dma_start(sg[:], d_sg)
            if not uniform:
                nc.scalar.dma_start(isp[:], d_isp)
                nc.scalar.dma_start(vrow[:], d_v)
            for t in range(1, 4):
                nc.scalar.dma_start(lps[t][:], d_lp[t])
            for t in range(4, NT):
                nc.sync.dma_start(lps[t][:], d_lp[t])
                if t == 5:
                    nc.sync.dma_start(r1[:, CW:BLOC], d_r1[:, CW:BLOC])
                    nc.sync.dma_start(r2[:, CW:BLOC], d_r2[:, CW:BLOC])

            for ch in range(NCH):
                csl = slice(ch * CW, (ch + 1) * CW)
                sfps = {}
                wps = {}

                def emit_out(k, ot, ws=wps):
                    wpair = ws[k // 2]
                    wsl = slice((k % 2) * CW, (k % 2) * CW + CW)
                    for h in range(2):
                        nc.tensor.matmul(
                            ot[:, h * CW:(h + 1) * CW],
                            lps[k][:, 512 + h * 128:512 + (h + 1) * 128],
                            wpair[:, wsl], start=(k == 0), stop=(k == NT - 1),
                            skip_group_check=True)

                ot = otp.tile([128, 2 * CW], F32, tag="ot")
                for t in range(NT):
                    p12 = pp.tile([128, 2 * CW], F32, tag="P12")
                    sb = sp.tile([128, CW], F32, tag="S")
                    lp = lps[t]
                    nc.tensor.matmul(p12[:, 0:CW], lp[:, 0:128], r1[:, csl],
                                     start=True, stop=True)
                    nc.tensor.matmul(p12[:, CW:2 * CW], lp[:, 128:256],
                                     r1[:, csl], start=True, stop=True)
                    nc.tensor.matmul(sb[:], lp[:, 256:384], r1[:, csl],
                                     start=True, stop=False)
                    nc.tensor.matmul(sb[:], lp[:, 384:512], r2[:, csl],
                                     start=False, stop=uniform)
                    if not uniform:
                        nc.tensor.matmul(sb[:], isp[:, t * 128:(t + 1) * 128],
                                         vrow[:, csl], start=False, stop=True)
                    if t >= KSH:
                        emit_out(t - KSH, ot)

                    # squares: ACT covers [0:XW], DVE copy+mult covers the
                    # last OFFC columns (keeps ACT under the PE pace)
                    u12 = u12p.tile([128, 2 * CW], F32, tag="u12")
                    nc.scalar.activation(u12[:, 0:XW], p12[:, 0:XW], AF.Square)
                    if t % 2 == 1 and t >= 3:
                        q = (t - 3) // 2
                        nc.scalar.activation(wps[q][:], sfps[q][:], AF.Exp)
                    cc = ccp.tile([128, OFFC], F32, tag="cc")
                    nc.vector.tensor_copy(cc[:], p12[:, XW:2 * CW])
                    nc.vector.tensor_mul(u12[:, XW:2 * CW], cc[:], cc[:])
                    # t1 = sgn*u1 + S (frees the S bank early), then
                    # sf = sgn*u2 + t1 on GpSimd (SBUF-only)
                    t1 = t1p.tile([128, CW], F32, tag="t1")
                    if t % 2 == 0:
                        sfps[t // 2] = sfp.tile([128, 2 * CW], F32, tag="sf",
                                                name=f"sf{ch}_{t // 2}")
                        wps[t // 2] = wp.tile([128, 2 * CW], BF16, tag="w",
                                              name=f"w{ch}_{t // 2}")
                    sfh = sfps[t // 2][:, (t % 2) * CW:(t % 2) * CW + CW]
                    if sgn_mode == 1:
                        nc.vector.tensor_add(t1[:], u12[:, 0:CW], sb[:])
                        nc.gpsimd.tensor_add(sfh, u12[:, CW:2 * CW], t1[:])
                    elif sgn_mode == -1:
                        nc.vector.scalar_tensor_tensor(
                            t1[:], u12[:, 0:CW], -1.0, sb[:],
                            op0=OP.mult, op1=OP.add)
                        nc.gpsimd.scalar_tensor_tensor(
                            sfh, u12[:, CW:2 * CW], -1.0, t1[:],
                            op0=OP.mult, op1=OP.add)
                    else:
                        nc.vector.scalar_tensor_tensor(
                            t1[:], u12[:, 0:CW], sg[:, t:t + 1], sb[:],
                            op0=OP.mult, op1=OP.add)
                        nc.gpsimd.scalar_tensor_tensor(
                            sfh, u12[:, CW:2 * CW], sg[:, t:t + 1], t1[:],
                            op0=OP.mult, op1=OP.add)

                # drain the per-chunk pipeline (pairs 0..(NT-3)//2 emitted
                # in-loop at odd tiles)
                nc.scalar.activation(wps[NT // 2 - 1][:],
                                     sfps[NT // 2 - 1][:], AF.Exp)
                for k in range(NT - KSH, NT):
                    emit_out(k, ot)

                obs = obsp.tile([128, 2 * CW], F32, tag="obs")
                nc.vector.tensor_copy(obs[:], ot[:])
                for h in range(2):
                    nc.sync.dma_start(d_out[h * 128:(h + 1) * 128, csl],
                                      obs[:, h * CW:(h + 1) * CW])
    nc.compile()
    _CACHED[key] = nc
    return nc


def _run(inputs, trace=False):
    from concourse.bass_utils import run_bass_kernel_spmd

    prep = _prep(**inputs)
    nc = _build_nc(prep["uniform"], prep["sgn_mode"])
    in_maps = []
    for c in range(NCORES):
        g, bg = c // BGN, c % BGN
        bsl = slice(bg * BLOC, (bg + 1) * BLOC)
        im = dict(r1=np.ascontiguousarray(prep["r1"][:, bsl]),
                  r2=np.ascontiguousarray(prep["r2"][:, bsl]),
                  lp=prep["lpack"][g])
        if prep["sgn_mode"] == 0:
            im["sg"] = np.ascontiguousarray(prep["sg"][g])
        if not prep["uniform"]:
            im["isp"] = np.ascontiguousarray(
                prep["ispk"][:, g * MLOC:(g + 1) * MLOC])
            im["vrow"] = np.ascontiguousarray(prep["vrow"][:, bsl])
        in_maps.append(im)
    res = run_bass_kernel_spmd(nc, in_maps, list(range(NCORES)), trace=trace)
    out = np.empty((B, S), np.float64)
    for bg in range(BGN):
        sm = (res.results[bg]["out"].astype(np.float64)
              + res.results[BGN + bg]["out"].astype(np.float64))
        bsl = slice(bg * BLOC, (bg + 1) * BLOC)
        out[bsl] = sm.T * prep["erow"][bsl, None]
    return out.astype(np.float32), res


def kernel(**inputs):
    out, _ = _run(inputs, trace=False)
    return out


def _install_ntff_hook():
    """The agent image's antenv lacks axon_hooks; recreate it so trace=True
    can capture NTFF profiles via libaxon_pjrt.so (same mechanism as
    trn_agent_boot.trn_boot)."""
    import types

    try:
        from antenv.axon_hooks import get_axon_ntff_profile_hook  # noqa: F401
        return
    except ImportError:
        pass
    import contextlib
    import ctypes

    so_path = "/opt/axon/libaxon_pjrt.so"
    lib = ctypes.CDLL(so_path)
    lib.axon_start_nrt_profile.argtypes = [ctypes.POINTER(ctypes.c_int64),
                                           ctypes.c_size_t]
    lib.axon_start_nrt_profile.restype = ctypes.c_int64
    lib.axon_stop_nrt_profile.argtypes = [ctypes.c_char_p]
    lib.axon_stop_nrt_profile.restype = ctypes.c_int64

    @contextlib.contextmanager
    def _hook(output_dir, device_ids):
        import jax

        jax.devices()
        if device_ids:
            ids = (ctypes.c_int64 * len(device_ids))(*device_ids)
            rc = lib.axon_start_nrt_profile(ids, len(device_ids))
        else:
            rc = lib.axon_start_nrt_profile(None, 0)
        if rc != 0:
            raise RuntimeError(f"axon_start_nrt_profile rc={rc}")
        try:
            yield
        finally:
            n = lib.axon_stop_nrt_profile(str(output_dir).encode())
            if n < 0:
                raise RuntimeError(f"axon_stop_nrt_profile rc={n}")
            if n == 0:
                print("WARNING: NTFF capture wrote nothing (raced the execute)")

    mod = types.ModuleType("antenv.axon_hooks")
    mod.get_axon_ntff_profile_hook = lambda: _hook
    mod.set_axon_ntff_profile_hook = lambda h: None
    sys.modules["antenv.axon_hooks"] = mod
    import antenv

    antenv.axon_hooks = mod


def run_traced(inputs):
    _install_ntff_hook()
    return _run(inputs, trace=True)


# revision 25
# speedup vs baseline: 1.2873x; 1.0602x over previous
"""CPSF codebook fused kernel for 8 Trainium2 NeuronCores.

Math (see reference): for each batch row b and codebook entry m,
  q[b,m] = par_sq/s_par + (max(tot_sq-par_sq,0) + max(dd_sq,0))/s_perp
  w[b,m] = alpha[m] * exp(-pi*q)
  out    = Re((w @ (T_hat_re + i*T_hat_im)) @ A.T),  A = exp(i*2pi/S * k*s)

Device strategy (M x B grid over 8 cores, host-side reduction):
  - The final DFT is folded into the codebook on the host:
      out = w @ TA,  TA = T_hat_re @ cos(ang) - T_hat_im @ sin(ang)
  - ALL linear and constant terms of the exponent are folded into the
    codebook-side matmul stacks on the host, so the device evaluates
      w[b,m] = exp( sgn*(M1^2 + M2^2) + S + be[m] )
    with M1/M2/S raw matmul outputs. Per 128m x 512b tile that is:
    4 weight matmuls + 2 out matmuls (PE), one Square (ACT), one
    self-multiply (DVE), one add (GpSimd), one PSUM-combine (DVE),
    one Exp (ACT) -- every engine under the PE's ~1.4us/tile pace.
  - Cores form a 2x4 grid: m-half x b-quarter. Each core computes
    partial out.T[256, 1024] for its quarter of the batch over half the
    codebook; the host adds the two m-halves, transposes, and applies
    the per-row exp factor. The relu clamps are dropped: tot_sq-par_sq
    >= 0 and dd_sq >= 0 hold mathematically; clamping only trims float
    roundoff ~1e-6, far below output noise.
"""

import os
import sys

for _p in ("/opt/trn_rl_repo", os.path.expanduser("~/.axon_site/_ro/trn_rl_repo")):
    if os.path.isdir(_p) and _p not in sys.path:
        sys.path.insert(0, _p)

import numpy as np

B, N, M, S = 4096, 64, 8192, 256
NCORES = 8
MG = 2                      # m-groups (codebook halves)
BGN = 4                     # b-groups (batch quarters)
MLOC = M // MG              # 4096 codebook entries per core
NT = MLOC // 128            # 32 m-tiles per core
BLOC = B // BGN             # 1024 batch rows per core
CW = 512                    # PSUM tile width (batch cols per chunk)
NCH = BLOC // CW            # 2 chunks per core
KSH = 6                     # out-matmul software pipeline shift (tiles)
PI = float(np.pi)

OFFC = 96                   # square columns offloaded from ACT to DVE copy+mult
XW = 2 * CW - OFFC          # ACT square width


def _prep(x_re, x_im, z_j_re, z_j_im, vec_d_j_re, vec_d_j_im,
          T_hat_re, T_hat_im, alpha_j, sigma_par, sigma_perp):
    """Host-side operand packing (all O(B*N + M*N + M*S^2) — tiny vs device)."""
    f32 = np.float32
    f64 = np.float64
    tiny = np.finfo(f32).tiny

    # ---- batch side ----
    z_re = np.ascontiguousarray(x_re[:, :N]).astype(f32)
    z_im = np.ascontiguousarray(x_im[:, :N]).astype(f32)
    vd_re = np.ascontiguousarray(x_re[:, N:]).astype(f32)
    vd_im = np.ascontiguousarray(x_im[:, N:]).astype(f32)
    nrm = np.sqrt((vd_re * vd_re + vd_im * vd_im).sum(-1, dtype=f32)).astype(f32)
    nrm = np.where(nrm == 0, f32(1.0), nrm)
    vd_re = vd_re / nrm[:, None]
    vd_im = vd_im / nrm[:, None]
    z_sq = (z_re * z_re + z_im * z_im).sum(-1, dtype=f32)
    vd_sq = (vd_re * vd_re + vd_im * vd_im).sum(-1, dtype=f32)

    import ml_dtypes
    bf = ml_dtypes.bfloat16
    # batch-side operands in bf16 to match the bf16 codebook stacks
    # (matmul inputs must be same width)
    r1 = np.ascontiguousarray(np.concatenate([z_re.T, z_im.T], 0)).astype(bf)
    r2 = np.ascontiguousarray(np.concatenate([vd_re.T, vd_im.T], 0)).astype(bf)

    # ---- codebook side ----
    djr = z_j_re.astype(f32)
    dji = z_j_im.astype(f32)
    vr = vec_d_j_re.astype(f32)
    vi = vec_d_j_im.astype(f32)
    nj = np.sqrt((vr * vr + vi * vi).sum(-1, dtype=f32)).astype(f32)
    nj = np.where(nj == 0, f32(1.0), nj)
    vr = vr / nj[:, None]
    vi = vi / nj[:, None]

    alpha = np.maximum(alpha_j.astype(f32), tiny).astype(f64)
    s_par = np.maximum(sigma_par.astype(f32), tiny).astype(f64)
    s_perp = np.maximum(sigma_perp.astype(f32), tiny).astype(f64)
    inv_sp = 1.0 / s_perp
    dbar = inv_sp - 1.0 / s_par
    gam = np.sqrt(PI * np.abs(dbar))
    sgn = np.where(dbar >= 0, 1.0, -1.0)

    c0_re = ((vr * djr + vi * dji).astype(f64)).sum(-1)
    c0_im = ((vr * dji - vi * djr).astype(f64)).sum(-1)
    z_j_sq = ((djr * djr + dji * dji).astype(f64)).sum(-1)
    vd_j_sq = ((vr * vr + vi * vi).astype(f64)).sum(-1)

    a1 = np.concatenate([vr.T, vi.T], 0).astype(f64)      # [128, M]
    a2 = np.concatenate([-vi.T, vr.T], 0).astype(f64)
    zc = np.concatenate([djr.T, dji.T], 0).astype(f64)

    L1 = (gam[None, :] * a1).astype(f32)
    L2 = (gam[None, :] * a2).astype(f32)
    L3 = ((2.0 * PI * inv_sp)[None, :] * zc
          - (2.0 * PI * dbar)[None, :] * (c0_re[None, :] * a1
                                          + c0_im[None, :] * a2)).astype(f32)
    L4 = ((2.0 * PI * inv_sp)[None, :] * a1).astype(f32)
    # be is folded into the out-side codebook: TA' = e^be * TA, so the
    # device exp needs no bias and can run on two-tile [128,1024] pairs.
    be = (np.log(alpha) - PI * inv_sp * (z_j_sq + vd_j_sq)
          + PI * dbar * (c0_re * c0_re + c0_im * c0_im))

    # sgn mode: 1 => plain add; -1 => stt with -1; 0 => per-m AP
    if np.all(sgn > 0):
        sgn_mode = 1
    elif np.all(sgn < 0):
        sgn_mode = -1
    else:
        sgn_mode = 0

    # v[b] term: inv_sp = c0v + delta; c0v*v[b] factors out of the exp as a
    # per-b output row scale (exact); a nonzero delta needs a rank-1 matmul.
    # Threshold absorbs mean-rounding dust (exp-arg error < 1e-12).
    c0v = float(inv_sp.mean())
    delta = (inv_sp - c0v).astype(f32)
    uniform = bool(np.abs(delta).max() <= 1e-9 * abs(c0v))
    vraw = (z_sq + vd_sq).astype(f64)
    erow = np.exp(-PI * c0v * vraw)                       # [B] f64 row scale
    vrow = (z_sq + vd_sq).astype(f32)[None, :]            # [1, B]
    ispk = (-PI * delta).astype(f32)[None, :]             # [1, M]

    # DFT folded into the codebook (same fp32 angles as reference), then
    # scaled by e^be per row.
    nn = np.arange(S, dtype=f32)
    ang = f32(2.0 * np.pi / S) * (nn[:, None] * nn[None, :])
    cosA = np.cos(ang).astype(f32)
    sinA = np.sin(ang).astype(f32)
    TA = ((T_hat_re.astype(f64) @ cosA.astype(f64)
           - T_hat_im.astype(f64) @ sinA.astype(f64))
          * np.exp(be)[:, None]).astype(f32)

    # pack per m-group/m-tile: lpack [MG, NT, 128, 768] = [L1|L2|L3|L4|TA'],
    # stored bf16 (halves LDWEIGHTS + DMA; ~3e-3 output rel err, gate 2e-2)
    lpack = np.empty((MG, NT, 128, 768), bf)
    sg = np.empty((MG, 128, NT), f32)
    for g in range(MG):
        for t in range(NT):
            sl = slice(g * MLOC + t * 128, g * MLOC + (t + 1) * 128)
            lpack[g, t, :, 0:128] = L1[:, sl]
            lpack[g, t, :, 128:256] = L2[:, sl]
            lpack[g, t, :, 256:384] = L3[:, sl]
            lpack[g, t, :, 384:512] = L4[:, sl]
            lpack[g, t, :, 512:768] = TA[sl]
            sg[g, :, t] = sgn[sl].astype(f32)

    return dict(r1=r1, r2=r2, lpack=lpack, sg=sg, vrow=vrow,
                ispk=ispk, erow=erow, uniform=uniform, sgn_mode=sgn_mode)


_CACHED = {}


def _build_nc(uniform, sgn_mode):
    key = ("nc", uniform, sgn_mode)
    if key in _CACHED:
        return _CACHED[key]
    import concourse.bacc as bacc
    import concourse.mybir as mybir
    import concourse.tile as tile

    F32 = mybir.dt.float32
    F32R = mybir.dt.float32r
    BF16 = mybir.dt.bfloat16
    AF = mybir.ActivationFunctionType
    OP = mybir.AluOpType

    nc = bacc.Bacc("TRN2", target_bir_lowering=False, debug=False,
                   num_devices=NCORES)
    d_r1 = nc.dram_tensor("r1", [128, BLOC], BF16, kind="ExternalInput").ap()
    d_r2 = nc.dram_tensor("r2", [128, BLOC], BF16, kind="ExternalInput").ap()
    d_lp = nc.dram_tensor("lp", [NT, 128, 768], BF16, kind="ExternalInput").ap()
    if sgn_mode == 0:
        d_sg = nc.dram_tensor("sg", [128, NT], F32, kind="ExternalInput").ap()
    if not uniform:
        d_isp = nc.dram_tensor("isp", [1, MLOC], F32R, kind="ExternalInput").ap()
        d_v = nc.dram_tensor("vrow", [1, BLOC], F32R, kind="ExternalInput").ap()
    d_out = nc.dram_tensor("out", [256, BLOC], F32, kind="ExternalOutput").ap()

    with tile.TileContext(nc) as tc:
        with tc.tile_pool(name="const", bufs=1) as cp, \
             tc.tile_pool(name="pp", bufs=2, space="PSUM") as pp, \
             tc.tile_pool(name="sp", bufs=2, space="PSUM") as sp, \
             tc.tile_pool(name="otp", bufs=1, space="PSUM") as otp, \
             tc.tile_pool(name="u12p", bufs=3) as u12p, \
             tc.tile_pool(name="ccp", bufs=3) as ccp, \
             tc.tile_pool(name="t1p", bufs=3) as t1p, \
             tc.tile_pool(name="sfp", bufs=3) as sfp, \
             tc.tile_pool(name="wp", bufs=4) as wp, \
             tc.tile_pool(name="obsp", bufs=2) as obsp:
            r1 = cp.tile([128, BLOC], BF16)
            r2 = cp.tile([128, BLOC], BF16)
            lps = [cp.tile([128, 768], BF16, name=f"lp{t}") for t in range(NT)]
            if sgn_mode == 0:
                sg = cp.tile([128, NT], F32)
            if not uniform:
                isp = cp.tile([1, MLOC], F32R)
                vrow = cp.tile([1, BLOC], F32R)

            # initial DMAs split across the two HWDGE queues (sync + scalar;
            # scalar is idle until the first Square) so the first tile's
            # operands land fast
            nc.sync.dma_start(lps[0][:], d_lp[0])
            nc.scalar.dma_start(r1[:, 0:CW], d_r1[:, 0:CW])
            nc.sync.dma_start(r2[:, 0:CW], d_r2[:, 0:CW])
            if sgn_mode == 0:
                nc.scalar.dma_start(sg[:], d_sg)
            if not uniform:
                nc.scalar.dma_start(isp[:], d_isp)
                nc.scalar.dma_start(vrow[:], d_v)
            for t in range(1, 4):
                nc.scalar.dma_start(lps[t][:], d_lp[t])
            for t in range(4, NT):
                nc.sync.dma_start(lps[t][:], d_lp[t])
                if t == 5:
                    nc.sync.dma_start(r1[:, CW:BLOC], d_r1[:, CW:BLOC])
                    nc.sync.dma_start(r2[:, CW:BLOC], d_r2[:, CW:BLOC])

            for ch in range(NCH):
                csl = slice(ch * CW, (ch + 1) * CW)
                sfps = {}
                wps = {}

                def emit_out(k, ot, ws=wps):
                    wpair = ws[k // 2]
                    wsl = slice((k % 2) * CW, (k % 2) * CW + CW)
                    for h in range(2):
                        nc.tensor.matmul(
                            ot[:, h * CW:(h + 1) * CW],
                            lps[k][:, 512 + h * 128:512 + (h + 1) * 128],
                            wpair[:, wsl], start=(k == 0), stop=(k == NT - 1),
                            skip_group_check=True)

                ot = otp.tile([128, 2 * CW], F32, tag="ot")
                for t in range(NT):
                    p12 = pp.tile([128, 2 * CW], F32, tag="P12")
                    sb = sp.tile([128, CW], F32, tag="S")
                    lp = lps[t]
                    nc.tensor.matmul(p12[:, 0:CW], lp[:, 0:128], r1[:, csl],
                                     start=True, stop=True)
                    nc.tensor.matmul(p12[:, CW:2 * CW], lp[:, 128:256],
                                     r1[:, csl], start=True, stop=True)
                    nc.tensor.matmul(sb[:], lp[:, 256:384], r1[:, csl],
                                     start=True, stop=False)
                    nc.tensor.matmul(sb[:], lp[:, 384:512], r2[:, csl],
                                     start=False, stop=uniform)
                    if not uniform:
                        nc.tensor.matmul(sb[:], isp[:, t * 128:(t + 1) * 128],
                                         vrow[:, csl], start=False, stop=True)
                    if t >= KSH:
                        emit_out(t - KSH, ot)

                    # squares: ACT covers [0:XW], DVE copy+mult covers the
                    # last OFFC columns (keeps ACT under the PE pace)
                    u12 = u12p.tile([128, 2 * CW], F32, tag="u12")
                    nc.scalar.activation(u12[:, 0:XW], p12[:, 0:XW], AF.Square)
                    if t % 2 == 1 and t >= 3:
                        q = (t - 3) // 2
                        nc.scalar.activation(wps[q][:], sfps[q][:], AF.Exp)
                    cc = ccp.tile([128, OFFC], F32, tag="cc")
                    nc.vector.tensor_copy(cc[:], p12[:, XW:2 * CW])
                    nc.vector.tensor_mul(u12[:, XW:2 * CW], cc[:], cc[:])
                    # t1 = sgn*u1 + S (frees the S bank early), then
                    # sf = sgn*u2 + t1 on GpSimd (SBUF-only)
                    t1 = t1p.tile([128, CW], F32, tag="t1")
                    if t % 2 == 0:
                        sfps[t // 2] = sfp.tile([128, 2 * CW], F32, tag="sf",
                                                name=f"sf{ch}_{t // 2}")
                        wps[t // 2] = wp.tile([128, 2 * CW], BF16, tag="w",
                                              name=f"w{ch}_{t // 2}")
                    sfh = sfps[t // 2][:, (t % 2) * CW:(t % 2) * CW + CW]
                    if sgn_mode == 1:
                        nc.vector.tensor_add(t1[:], u12[:, 0:CW], sb[:])
                        nc.gpsimd.tensor_add(sfh, u12[:, CW:2 * CW], t1[:])
                    elif sgn_mode == -1:
                        nc.vector.scalar_tensor_tensor(
                            t1[:], u12[:, 0:CW], -1.0, sb[:],
                            op0=OP.mult, op1=OP.add)
                        nc.gpsimd.scalar_tensor_tensor(
                            sfh, u12[:, CW:2 * CW], -1.0, t1[:],
                            op0=OP.mult, op1=OP.add)
                    else:
                        nc.vector.scalar_tensor_tensor(
                            t1[:], u12[:, 0:CW], sg[:, t:t + 1], sb[:],
                            op0=OP.mult, op1=OP.add)
                        nc.gpsimd.scalar_tensor_tensor(
                            sfh, u12[:, CW:2 * CW], sg[:, t:t + 1], t1[:],
                            op0=OP.mult, op1=OP.add)

                # drain the per-chunk pipeline (pairs 0..(NT-3)//2 emitted
                # in-loop at odd tiles)
                nc.scalar.activation(wps[NT // 2 - 1][:],
                                     sfps[NT // 2 - 1][:], AF.Exp)
                for k in range(NT - KSH, NT):
                    emit_out(k, ot)

                obs = obsp.tile([128, 2 * CW], F32, tag="obs")
                nc.scalar.copy(obs[:, 0:CW], ot[:, 0:CW])
                nc.vector.tensor_copy(obs[:, CW:2 * CW], ot[:, CW:2 * CW])
                nc.sync.dma_start(d_out[0:128, csl], obs[:, 0:CW])
                nc.scalar.dma_start(d_out[128:256, csl], obs[:, CW:2 * CW])
    nc.compile()
    _CACHED[key] = nc
    return nc


def _run(inputs, trace=False):
    from concourse.bass_utils import run_bass_kernel_spmd

    prep = _prep(**inputs)
    nc = _build_nc(prep["uniform"], prep["sgn_mode"])
    in_maps = []
    for c in range(NCORES):
        g, bg = c // BGN, c % BGN
        bsl = slice(bg * BLOC, (bg + 1) * BLOC)
        im = dict(r1=np.ascontiguousarray(prep["r1"][:, bsl]),
                  r2=np.ascontiguousarray(prep["r2"][:, bsl]),
                  lp=prep["lpack"][g])
        if prep["sgn_mode"] == 0:
            im["sg"] = np.ascontiguousarray(prep["sg"][g])
        if not prep["uniform"]:
            im["isp"] = np.ascontiguousarray(
                prep["ispk"][:, g * MLOC:(g + 1) * MLOC])
            im["vrow"] = np.ascontiguousarray(prep["vrow"][:, bsl])
        in_maps.append(im)
    res = run_bass_kernel_spmd(nc, in_maps, list(range(NCORES)), trace=trace)
    out = np.empty((B, S), np.float64)
    for bg in range(BGN):
        sm = (res.results[bg]["out"].astype(np.float64)
              + res.results[BGN + bg]["out"].astype(np.float64))
        bsl = slice(bg * BLOC, (bg + 1) * BLOC)
        out[bsl] = sm.T * prep["erow"][bsl, None]
    return out.astype(np.float32), res


def kernel(**inputs):
    out, _ = _run(inputs, trace=False)
    return out


def _install_ntff_hook():
    """The agent image's antenv lacks axon_hooks; recreate it so trace=True
    can capture NTFF profiles via libaxon_pjrt.so (same mechanism as
    trn_agent_boot.trn_boot)."""
    import types

    try:
        from antenv.axon_hooks import get_axon_ntff_profile_hook  # noqa: F401
        return
    except ImportError:
        pass
    import contextlib
    import ctypes

    so_path = "/opt/axon/libaxon_pjrt.so"
    lib = ctypes.CDLL(so_path)
    lib.axon_start_nrt_profile.argtypes = [ctypes.POINTER(ctypes.c_int64),
                                           ctypes.c_size_t]
    lib.axon_start_nrt_profile.restype = ctypes.c_int64
    lib.axon_stop_nrt_profile.argtypes = [ctypes.c_char_p]
    lib.axon_stop_nrt_profile.restype = ctypes.c_int64

    @contextlib.contextmanager
    def _hook(output_dir, device_ids):
        import jax

        jax.devices()
        if device_ids:
            ids = (ctypes.c_int64 * len(device_ids))(*device_ids)
            rc = lib.axon_start_nrt_profile(ids, len(device_ids))
        else:
            rc = lib.axon_start_nrt_profile(None, 0)
        if rc != 0:
            raise RuntimeError(f"axon_start_nrt_profile rc={rc}")
        try:
            yield
        finally:
            n = lib.axon_stop_nrt_profile(str(output_dir).encode())
            if n < 0:
                raise RuntimeError(f"axon_stop_nrt_profile rc={n}")
            if n == 0:
                print("WARNING: NTFF capture wrote nothing (raced the execute)")

    mod = types.ModuleType("antenv.axon_hooks")
    mod.get_axon_ntff_profile_hook = lambda: _hook
    mod.set_axon_ntff_profile_hook = lambda h: None
    sys.modules["antenv.axon_hooks"] = mod
    import antenv

    antenv.axon_hooks = mod


def run_traced(inputs):
    _install_ntff_hook()
    return _run(inputs, trace=True)


# revision 26
# speedup vs baseline: 1.3078x; 1.0160x over previous
"""CPSF codebook fused kernel for 8 Trainium2 NeuronCores.

Math (see reference): for each batch row b and codebook entry m,
  q[b,m] = par_sq/s_par + (max(tot_sq-par_sq,0) + max(dd_sq,0))/s_perp
  w[b,m] = alpha[m] * exp(-pi*q)
  out    = Re((w @ (T_hat_re + i*T_hat_im)) @ A.T),  A = exp(i*2pi/S * k*s)

Device strategy (M x B grid over 8 cores, host-side reduction):
  - The final DFT is folded into the codebook on the host:
      out = w @ TA,  TA = T_hat_re @ cos(ang) - T_hat_im @ sin(ang)
  - ALL linear and constant terms of the exponent are folded into the
    codebook-side matmul stacks on the host, so the device evaluates
      w[b,m] = exp( sgn*(M1^2 + M2^2) + S + be[m] )
    with M1/M2/S raw matmul outputs. Per 128m x 512b tile that is:
    4 weight matmuls + 2 out matmuls (PE), one Square (ACT), one
    self-multiply (DVE), one add (GpSimd), one PSUM-combine (DVE),
    one Exp (ACT) -- every engine under the PE's ~1.4us/tile pace.
  - Cores form a 2x4 grid: m-half x b-quarter. Each core computes
    partial out.T[256, 1024] for its quarter of the batch over half the
    codebook; the host adds the two m-halves, transposes, and applies
    the per-row exp factor. The relu clamps are dropped: tot_sq-par_sq
    >= 0 and dd_sq >= 0 hold mathematically; clamping only trims float
    roundoff ~1e-6, far below output noise.
"""

import os
import sys

for _p in ("/opt/trn_rl_repo", os.path.expanduser("~/.axon_site/_ro/trn_rl_repo")):
    if os.path.isdir(_p) and _p not in sys.path:
        sys.path.insert(0, _p)

import numpy as np

B, N, M, S = 4096, 64, 8192, 256
NCORES = 8
MG = 2                      # m-groups (codebook halves)
BGN = 4                     # b-groups (batch quarters)
MLOC = M // MG              # 4096 codebook entries per core
NT = MLOC // 128            # 32 m-tiles per core
BLOC = B // BGN             # 1024 batch rows per core
CW = 512                    # PSUM tile width (batch cols per chunk)
NCH = BLOC // CW            # 2 chunks per core
KSH = 6                     # out-matmul software pipeline shift (tiles)
PI = float(np.pi)

OFFC = 96                   # square columns offloaded from ACT to DVE copy+mult
XW = 2 * CW - OFFC          # ACT square width


def _prep(x_re, x_im, z_j_re, z_j_im, vec_d_j_re, vec_d_j_im,
          T_hat_re, T_hat_im, alpha_j, sigma_par, sigma_perp):
    """Host-side operand packing (all O(B*N + M*N + M*S^2) — tiny vs device)."""
    f32 = np.float32
    f64 = np.float64
    tiny = np.finfo(f32).tiny

    # ---- batch side ----
    z_re = np.ascontiguousarray(x_re[:, :N]).astype(f32)
    z_im = np.ascontiguousarray(x_im[:, :N]).astype(f32)
    vd_re = np.ascontiguousarray(x_re[:, N:]).astype(f32)
    vd_im = np.ascontiguousarray(x_im[:, N:]).astype(f32)
    nrm = np.sqrt((vd_re * vd_re + vd_im * vd_im).sum(-1, dtype=f32)).astype(f32)
    nrm = np.where(nrm == 0, f32(1.0), nrm)
    vd_re = vd_re / nrm[:, None]
    vd_im = vd_im / nrm[:, None]
    z_sq = (z_re * z_re + z_im * z_im).sum(-1, dtype=f32)
    vd_sq = (vd_re * vd_re + vd_im * vd_im).sum(-1, dtype=f32)

    import ml_dtypes
    bf = ml_dtypes.bfloat16
    # batch-side operands in bf16 to match the bf16 codebook stacks
    # (matmul inputs must be same width)
    r1 = np.ascontiguousarray(np.concatenate([z_re.T, z_im.T], 0)).astype(bf)
    r2 = np.ascontiguousarray(np.concatenate([vd_re.T, vd_im.T], 0)).astype(bf)

    # ---- codebook side ----
    djr = z_j_re.astype(f32)
    dji = z_j_im.astype(f32)
    vr = vec_d_j_re.astype(f32)
    vi = vec_d_j_im.astype(f32)
    nj = np.sqrt((vr * vr + vi * vi).sum(-1, dtype=f32)).astype(f32)
    nj = np.where(nj == 0, f32(1.0), nj)
    vr = vr / nj[:, None]
    vi = vi / nj[:, None]

    alpha = np.maximum(alpha_j.astype(f32), tiny).astype(f64)
    s_par = np.maximum(sigma_par.astype(f32), tiny).astype(f64)
    s_perp = np.maximum(sigma_perp.astype(f32), tiny).astype(f64)
    inv_sp = 1.0 / s_perp
    dbar = inv_sp - 1.0 / s_par
    gam = np.sqrt(PI * np.abs(dbar))
    sgn = np.where(dbar >= 0, 1.0, -1.0)

    c0_re = ((vr * djr + vi * dji).astype(f64)).sum(-1)
    c0_im = ((vr * dji - vi * djr).astype(f64)).sum(-1)
    z_j_sq = ((djr * djr + dji * dji).astype(f64)).sum(-1)
    vd_j_sq = ((vr * vr + vi * vi).astype(f64)).sum(-1)

    a1 = np.concatenate([vr.T, vi.T], 0).astype(f64)      # [128, M]
    a2 = np.concatenate([-vi.T, vr.T], 0).astype(f64)
    zc = np.concatenate([djr.T, dji.T], 0).astype(f64)

    L1 = (gam[None, :] * a1).astype(f32)
    L2 = (gam[None, :] * a2).astype(f32)
    L3 = ((2.0 * PI * inv_sp)[None, :] * zc
          - (2.0 * PI * dbar)[None, :] * (c0_re[None, :] * a1
                                          + c0_im[None, :] * a2)).astype(f32)
    L4 = ((2.0 * PI * inv_sp)[None, :] * a1).astype(f32)
    # be is folded into the out-side codebook: TA' = e^be * TA, so the
    # device exp needs no bias and can run on two-tile [128,1024] pairs.
    be = (np.log(alpha) - PI * inv_sp * (z_j_sq + vd_j_sq)
          + PI * dbar * (c0_re * c0_re + c0_im * c0_im))

    # sgn mode: 1 => plain add; -1 => stt with -1; 0 => per-m AP
    if np.all(sgn > 0):
        sgn_mode = 1
    elif np.all(sgn < 0):
        sgn_mode = -1
    else:
        sgn_mode = 0

    # v[b] term: inv_sp = c0v + delta; c0v*v[b] factors out of the exp as a
    # per-b output row scale (exact); a nonzero delta needs a rank-1 matmul.
    # Threshold absorbs mean-rounding dust (exp-arg error < 1e-12).
    c0v = float(inv_sp.mean())
    delta = (inv_sp - c0v).astype(f32)
    uniform = bool(np.abs(delta).max() <= 1e-9 * abs(c0v))
    vraw = (z_sq + vd_sq).astype(f64)
    erow = np.exp(-PI * c0v * vraw)                       # [B] f64 row scale
    vrow = (z_sq + vd_sq).astype(f32)[None, :]            # [1, B]
    ispk = (-PI * delta).astype(f32)[None, :]             # [1, M]

    # DFT folded into the codebook (same fp32 angles as reference), then
    # scaled by e^be per row.
    nn = np.arange(S, dtype=f32)
    ang = f32(2.0 * np.pi / S) * (nn[:, None] * nn[None, :])
    cosA = np.cos(ang).astype(f32)
    sinA = np.sin(ang).astype(f32)
    TA = ((T_hat_re.astype(f64) @ cosA.astype(f64)
           - T_hat_im.astype(f64) @ sinA.astype(f64))
          * np.exp(be)[:, None]).astype(f32)

    # pack per m-group/m-tile: lpack [MG, NT, 128, 768] = [L1|L2|L3|L4|TA'],
    # stored bf16 (halves LDWEIGHTS + DMA; ~3e-3 output rel err, gate 2e-2)
    lpack = np.empty((MG, NT, 128, 768), bf)
    sg = np.empty((MG, 128, NT), f32)
    for g in range(MG):
        for t in range(NT):
            sl = slice(g * MLOC + t * 128, g * MLOC + (t + 1) * 128)
            lpack[g, t, :, 0:128] = L1[:, sl]
            lpack[g, t, :, 128:256] = L2[:, sl]
            lpack[g, t, :, 256:384] = L3[:, sl]
            lpack[g, t, :, 384:512] = L4[:, sl]
            lpack[g, t, :, 512:768] = TA[sl]
            sg[g, :, t] = sgn[sl].astype(f32)

    return dict(r1=r1, r2=r2, lpack=lpack, sg=sg, vrow=vrow,
                ispk=ispk, erow=erow, uniform=uniform, sgn_mode=sgn_mode)


_CACHED = {}


def _build_nc(uniform, sgn_mode):
    key = ("nc", uniform, sgn_mode)
    if key in _CACHED:
        return _CACHED[key]
    import concourse.bacc as bacc
    import concourse.mybir as mybir
    import concourse.tile as tile

    F32 = mybir.dt.float32
    F32R = mybir.dt.float32r
    BF16 = mybir.dt.bfloat16
    AF = mybir.ActivationFunctionType
    OP = mybir.AluOpType

    nc = bacc.Bacc("TRN2", target_bir_lowering=False, debug=False,
                   num_devices=NCORES)
    d_r1 = nc.dram_tensor("r1", [128, BLOC], BF16, kind="ExternalInput").ap()
    d_r2 = nc.dram_tensor("r2", [128, BLOC], BF16, kind="ExternalInput").ap()
    d_lp = nc.dram_tensor("lp", [NT, 128, 768], BF16, kind="ExternalInput").ap()
    if sgn_mode == 0:
        d_sg = nc.dram_tensor("sg", [128, NT], F32, kind="ExternalInput").ap()
    if not uniform:
        d_isp = nc.dram_tensor("isp", [1, MLOC], F32R, kind="ExternalInput").ap()
        d_v = nc.dram_tensor("vrow", [1, BLOC], F32R, kind="ExternalInput").ap()
    d_out = nc.dram_tensor("out", [256, BLOC], F32, kind="ExternalOutput").ap()

    with tile.TileContext(nc) as tc:
        with tc.tile_pool(name="const", bufs=1) as cp, \
             tc.tile_pool(name="pp", bufs=2, space="PSUM") as pp, \
             tc.tile_pool(name="sp", bufs=2, space="PSUM") as sp, \
             tc.tile_pool(name="otp", bufs=1, space="PSUM") as otp, \
             tc.tile_pool(name="u12p", bufs=3) as u12p, \
             tc.tile_pool(name="ccp", bufs=3) as ccp, \
             tc.tile_pool(name="t1p", bufs=3) as t1p, \
             tc.tile_pool(name="sfp", bufs=3) as sfp, \
             tc.tile_pool(name="wp", bufs=4) as wp, \
             tc.tile_pool(name="obsp", bufs=2) as obsp:
            r1 = cp.tile([128, BLOC], BF16)
            r2 = cp.tile([128, BLOC], BF16)
            lps = [cp.tile([128, 768], BF16, name=f"lp{t}") for t in range(NT)]
            if sgn_mode == 0:
                sg = cp.tile([128, NT], F32)
            if not uniform:
                isp = cp.tile([1, MLOC], F32R)
                vrow = cp.tile([1, BLOC], F32R)

            # initial DMAs split across the two HWDGE queues (sync + scalar;
            # scalar is idle until the first Square) so the first tile's
            # operands land fast
            nc.sync.dma_start(lps[0][:], d_lp[0])
            nc.scalar.dma_start(r1[:, 0:CW], d_r1[:, 0:CW])
            nc.sync.dma_start(r2[:, 0:CW], d_r2[:, 0:CW])
            if sgn_mode == 0:
                nc.scalar.dma_start(sg[:], d_sg)
            if not uniform:
                nc.scalar.dma_start(isp[:], d_isp)
                nc.scalar.dma_start(vrow[:], d_v)
            for t in range(1, 4):
                nc.scalar.dma_start(lps[t][:], d_lp[t])
            for t in range(4, NT):
                nc.sync.dma_start(lps[t][:], d_lp[t])
                if t == 5:
                    nc.sync.dma_start(r1[:, CW:BLOC], d_r1[:, CW:BLOC])
                    nc.sync.dma_start(r2[:, CW:BLOC], d_r2[:, CW:BLOC])

            # one flat stream over (chunk, tile): the previous chunk's
            # pipelined OUT drain interleaves with the next chunk's leading
            # matmuls, hiding the chunk-boundary chain latency
            NK = NCH * NT
            sfps = {}
            wps = {}
            ots = {}

            def emit_evac(ch):
                csl = slice(ch * CW, (ch + 1) * CW)
                obs = obsp.tile([128, 2 * CW], F32, tag="obs",
                                name=f"obs{ch}")
                nc.scalar.copy(obs[:, 0:CW], ots[ch][:, 0:CW])
                nc.vector.tensor_copy(obs[:, CW:2 * CW],
                                      ots[ch][:, CW:2 * CW])
                nc.sync.dma_start(d_out[0:128, csl], obs[:, 0:CW])
                nc.scalar.dma_start(d_out[128:256, csl], obs[:, CW:2 * CW])

            def emit_out(j):
                wpair = wps[j // 2]
                wsl = slice((j % 2) * CW, (j % 2) * CW + CW)
                t = j % NT
                for h in range(2):
                    nc.tensor.matmul(
                        ots[j // NT][:, h * CW:(h + 1) * CW],
                        lps[t][:, 512 + h * 128:512 + (h + 1) * 128],
                        wpair[:, wsl], start=(t == 0), stop=(t == NT - 1),
                        skip_group_check=True)
                if t == NT - 1:
                    emit_evac(j // NT)

            for k in range(NK):
                ch, t = k // NT, k % NT
                csl = slice(ch * CW, (ch + 1) * CW)
                if t == 0:
                    ots[ch] = otp.tile([128, 2 * CW], F32, tag="ot",
                                       name=f"ot{ch}")
                p12 = pp.tile([128, 2 * CW], F32, tag="P12")
                sb = sp.tile([128, CW], F32, tag="S")
                lp = lps[t]
                nc.tensor.matmul(p12[:, 0:CW], lp[:, 0:128], r1[:, csl],
                                 start=True, stop=True)
                nc.tensor.matmul(p12[:, CW:2 * CW], lp[:, 128:256],
                                 r1[:, csl], start=True, stop=True)
                nc.tensor.matmul(sb[:], lp[:, 256:384], r1[:, csl],
                                 start=True, stop=False)
                nc.tensor.matmul(sb[:], lp[:, 384:512], r2[:, csl],
                                 start=False, stop=uniform)
                if not uniform:
                    nc.tensor.matmul(sb[:], isp[:, t * 128:(t + 1) * 128],
                                     vrow[:, csl], start=False, stop=True)
                if k >= KSH:
                    emit_out(k - KSH)

                # squares: ACT covers [0:XW], DVE copy+mult covers the
                # last OFFC columns (keeps ACT under the PE pace)
                u12 = u12p.tile([128, 2 * CW], F32, tag="u12")
                nc.scalar.activation(u12[:, 0:XW], p12[:, 0:XW], AF.Square)
                if k % 2 == 1 and k >= 3:
                    q = (k - 3) // 2
                    nc.scalar.activation(wps[q][:], sfps[q][:], AF.Exp)
                cc = ccp.tile([128, OFFC], F32, tag="cc")
                nc.vector.tensor_copy(cc[:], p12[:, XW:2 * CW])
                nc.vector.tensor_mul(u12[:, XW:2 * CW], cc[:], cc[:])
                # t1 = sgn*u1 + S (frees the S bank early), then
                # sf = sgn*u2 + t1 on GpSimd (SBUF-only)
                t1 = t1p.tile([128, CW], F32, tag="t1")
                if k % 2 == 0:
                    sfps[k // 2] = sfp.tile([128, 2 * CW], F32, tag="sf",
                                            name=f"sf{k // 2}")
                    wps[k // 2] = wp.tile([128, 2 * CW], BF16, tag="w",
                                          name=f"w{k // 2}")
                sfh = sfps[k // 2][:, (k % 2) * CW:(k % 2) * CW + CW]
                if sgn_mode == 1:
                    nc.vector.tensor_add(t1[:], u12[:, 0:CW], sb[:])
                    nc.gpsimd.tensor_add(sfh, u12[:, CW:2 * CW], t1[:])
                elif sgn_mode == -1:
                    nc.vector.scalar_tensor_tensor(
                        t1[:], u12[:, 0:CW], -1.0, sb[:],
                        op0=OP.mult, op1=OP.add)
                    nc.gpsimd.scalar_tensor_tensor(
                        sfh, u12[:, CW:2 * CW], -1.0, t1[:],
                        op0=OP.mult, op1=OP.add)
                else:
                    nc.vector.scalar_tensor_tensor(
                        t1[:], u12[:, 0:CW], sg[:, t:t + 1], sb[:],
                        op0=OP.mult, op1=OP.add)
                    nc.gpsimd.scalar_tensor_tensor(
                        sfh, u12[:, CW:2 * CW], sg[:, t:t + 1], t1[:],
                        op0=OP.mult, op1=OP.add)

            # drain (pairs 0..(NK-3)//2 emitted in-loop at odd k)
            nc.scalar.activation(wps[NK // 2 - 1][:],
                                 sfps[NK // 2 - 1][:], AF.Exp)
            for j in range(NK - KSH, NK):
                emit_out(j)
    nc.compile()
    _CACHED[key] = nc
    return nc


def _run(inputs, trace=False):
    from concourse.bass_utils import run_bass_kernel_spmd

    prep = _prep(**inputs)
    nc = _build_nc(prep["uniform"], prep["sgn_mode"])
    in_maps = []
    for c in range(NCORES):
        g, bg = c // BGN, c % BGN
        bsl = slice(bg * BLOC, (bg + 1) * BLOC)
        im = dict(r1=np.ascontiguousarray(prep["r1"][:, bsl]),
                  r2=np.ascontiguousarray(prep["r2"][:, bsl]),
                  lp=prep["lpack"][g])
        if prep["sgn_mode"] == 0:
            im["sg"] = np.ascontiguousarray(prep["sg"][g])
        if not prep["uniform"]:
            im["isp"] = np.ascontiguousarray(
                prep["ispk"][:, g * MLOC:(g + 1) * MLOC])
            im["vrow"] = np.ascontiguousarray(prep["vrow"][:, bsl])
        in_maps.append(im)
    res = run_bass_kernel_spmd(nc, in_maps, list(range(NCORES)), trace=trace)
    out = np.empty((B, S), np.float64)
    for bg in range(BGN):
        sm = (res.results[bg]["out"].astype(np.float64)
              + res.results[BGN + bg]["out"].astype(np.float64))
        bsl = slice(bg * BLOC, (bg + 1) * BLOC)
        out[bsl] = sm.T * prep["erow"][bsl, None]
    return out.astype(np.float32), res


def kernel(**inputs):
    out, _ = _run(inputs, trace=False)
    return out


def _install_ntff_hook():
    """The agent image's antenv lacks axon_hooks; recreate it so trace=True
    can capture NTFF profiles via libaxon_pjrt.so (same mechanism as
    trn_agent_boot.trn_boot)."""
    import types

    try:
        from antenv.axon_hooks import get_axon_ntff_profile_hook  # noqa: F401
        return
    except ImportError:
        pass
    import contextlib
    import ctypes

    so_path = "/opt/axon/libaxon_pjrt.so"
    lib = ctypes.CDLL(so_path)
    lib.axon_start_nrt_profile.argtypes = [ctypes.POINTER(ctypes.c_int64),
                                           ctypes.c_size_t]
    lib.axon_start_nrt_profile.restype = ctypes.c_int64
    lib.axon_stop_nrt_profile.argtypes = [ctypes.c_char_p]
    lib.axon_stop_nrt_profile.restype = ctypes.c_int64

    @contextlib.contextmanager
    def _hook(output_dir, device_ids):
        import jax

        jax.devices()
        if device_ids:
            ids = (ctypes.c_int64 * len(device_ids))(*device_ids)
            rc = lib.axon_start_nrt_profile(ids, len(device_ids))
        else:
            rc = lib.axon_start_nrt_profile(None, 0)
        if rc != 0:
            raise RuntimeError(f"axon_start_nrt_profile rc={rc}")
        try:
            yield
        finally:
            n = lib.axon_stop_nrt_profile(str(output_dir).encode())
            if n < 0:
                raise RuntimeError(f"axon_stop_nrt_profile rc={n}")
            if n == 0:
                print("WARNING: NTFF capture wrote nothing (raced the execute)")

    mod = types.ModuleType("antenv.axon_hooks")
    mod.get_axon_ntff_profile_hook = lambda: _hook
    mod.set_axon_ntff_profile_hook = lambda h: None
    sys.modules["antenv.axon_hooks"] = mod
    import antenv

    antenv.axon_hooks = mod


def run_traced(inputs):
    _install_ntff_hook()
    return _run(inputs, trace=True)
